# revision 29
# baseline (speedup 1.0000x reference)
"""Trainium2 Bass kernel: FFT low-pass detrend + linear trend forecast.

reference math:
    X = rfft(x); keep bins 0..4; trend = irfft(X_low); resid = x - trend
    fc = trend @ W.T + b

Keeping the lowest 5 rfft bins is an orthogonal rank-9 projection:
    trend = (x @ C) @ C.T          C: [512, 9] = [DC, cos/sin k=1..4]
    fc    = (x @ C) @ (C.T @ W.T) + b
so the whole module is two tiny matmuls + a subtract. No FFT on device.

Sharding: pure data parallel over the batch dim across 8 NeuronCores.
Per core: 8192 rows x 512. Blocks of 512 rows (4x128-row subtiles):
  load x -> cast bf16 -> TensorE transpose-mode (bf16 pairs packed in
  f32 words: half the transpose instructions) -> batched N=512 bf16
  projection matmuls -> bf16 reconstruct + forecast matmuls -> DVE
  subtract -> stores.
Loads ride the Sync HWDGE ring; stores ride GpSimd SWDGE so slow
stores don't head-of-line-block the next block's load.
"""

import numpy as np

SEQ = 512
PRED = 96
NBINS = 5
NB9 = 2 * NBINS - 1  # 9 basis functions
NCORES = 8
B, NCH = 64, 1024
ROWS_PER_CORE = B * NCH // NCORES  # 8192
CSUB = 4  # 128-row subtiles per block
BLK_ROWS = 128 * CSUB  # 512
NBLK = ROWS_PER_CORE // BLK_ROWS  # 16

_CACHE = {}


def _basis_f64():
    s = np.arange(SEQ, dtype=np.float64)
    cols = [np.full(SEQ, 1.0 / np.sqrt(SEQ))]
    for k in range(1, NBINS):
        w = 2.0 * np.pi * k * s / SEQ
        cols.append(np.sqrt(2.0 / SEQ) * np.cos(w))
        cols.append(np.sqrt(2.0 / SEQ) * np.sin(w))
    return np.stack(cols, axis=1)  # [SEQ, 9]


def _host_consts(W, b):
    import ml_dtypes

    C = _basis_f64()
    v = np.arange(128)
    # cproj[v, h, q, j9] = C[2*(128h+v)+q, j9]  (packed-pair transpose layout)
    cproj = np.zeros((128, 2, 2, NB9), dtype=np.float64)
    for h in range(2):
        for q in range(2):
            cproj[:, h, q, :] = C[2 * (128 * h + v) + q, :]
    g = C.T @ W.astype(np.float64).T  # [9, 96]
    return {
        "cproj": cproj.astype(ml_dtypes.bfloat16),
        "ctrec": np.ascontiguousarray(C.T).astype(ml_dtypes.bfloat16),
        "g": g.astype(ml_dtypes.bfloat16),
        "biasr": np.ascontiguousarray(
            np.broadcast_to(b.astype(np.float32), (128, CSUB, PRED))
        ),
        "ident": np.eye(128, dtype=np.float32),
    }


def _build_nc(nblk=NBLK):
    from contextlib import ExitStack

    import concourse.mybir as mybir
    import concourse.tile as tile
    from concourse import bacc

    f32 = mybir.dt.float32
    bf16 = mybir.dt.bfloat16

    rows = nblk * BLK_ROWS
    nc = bacc.Bacc("TRN2", target_bir_lowering=False, debug=False)
    x_d = nc.dram_tensor("x", [rows, SEQ], f32, kind="ExternalInput")
    cproj_d = nc.dram_tensor("cproj", [128, 2, 2, NB9], bf16, kind="ExternalInput")
    ctrec_d = nc.dram_tensor("ctrec", [NB9, SEQ], bf16, kind="ExternalInput")
    g_d = nc.dram_tensor("g", [NB9, PRED], bf16, kind="ExternalInput")
    biasr_d = nc.dram_tensor("biasr", [128, CSUB, PRED], f32, kind="ExternalInput")
    ident_d = nc.dram_tensor("ident", [128, 128], f32, kind="ExternalInput")
    fc_d = nc.dram_tensor("fc", [rows, PRED], f32, kind="ExternalOutput")
    resid_d = nc.dram_tensor("resid", [rows, SEQ], f32, kind="ExternalOutput")

    # row r = nb*512 + c*128 + p
    x_v = x_d.ap().rearrange("(nb c p) s -> nb c p s", c=CSUB, p=128)
    resid_v = resid_d.ap().rearrange("(nb c p) s -> nb c p s", c=CSUB, p=128)
    fc_v = fc_d.ap().rearrange("(nb c p) j -> nb p c j", c=CSUB, p=128)

    with ExitStack() as ctx:
        tc = ctx.enter_context(tile.TileContext(nc))
        consts = ctx.enter_context(tc.tile_pool(name="consts", bufs=1))
        xin = ctx.enter_context(tc.tile_pool(name="xin", bufs=5))
        xbp = ctx.enter_context(tc.tile_pool(name="xbp", bufs=3))
        xtp = ctx.enter_context(tc.tile_pool(name="xtp", bufs=3))
        cfp = ctx.enter_context(tc.tile_pool(name="cfp", bufs=3))
        rout = ctx.enter_context(tc.tile_pool(name="rout", bufs=3))
        fout = ctx.enter_context(tc.tile_pool(name="fout", bufs=3))
        # PSUM budget: 8 banks. xt 3x1 + trend 2x1 + coeff 2x1 + fc 1x1 = 8
        ps_xt = ctx.enter_context(tc.tile_pool(name="ps_xt", bufs=3, space="PSUM"))
        ps_tr = ctx.enter_context(tc.tile_pool(name="ps_tr", bufs=2, space="PSUM"))
        ps_cf = ctx.enter_context(tc.tile_pool(name="ps_cf", bufs=2, space="PSUM"))
        ps_fc = ctx.enter_context(tc.tile_pool(name="ps_fc", bufs=1, space="PSUM"))

        # consts ride GpSimd SWDGE so the first x loads start immediately on Sync
        ident = consts.tile([128, 128], f32)
        nc.gpsimd.dma_start(out=ident[:, :], in_=ident_d.ap())
        cproj = consts.tile([128, 2, 2, NB9], bf16)
        nc.gpsimd.dma_start(out=cproj[:, :, :, :], in_=cproj_d.ap())
        ctrec = consts.tile([NB9, SEQ], bf16)
        nc.gpsimd.dma_start(out=ctrec[:, :], in_=ctrec_d.ap())
        g_sb = consts.tile([NB9, PRED], bf16)
        nc.gpsimd.dma_start(out=g_sb[:, :], in_=g_d.ap())
        biasr = consts.tile([128, CSUB, PRED], f32)
        nc.gpsimd.dma_start(out=biasr[:, :, :], in_=biasr_d.ap())

        for nb in range(nblk):
            # loads: two 512KiB DMAs per block (block 0: four 256KiB to cut
            # the time-to-first-compute at kernel start)
            x_sb = xin.tile([128, CSUB, SEQ], f32, tag="x_sb")
            nld = 4 if nb == 0 else 2
            for dd in range(nld):
                w = CSUB // nld
                nc.sync.dma_start(
                    out=x_sb[:, w * dd : w * (dd + 1), :],
                    in_=x_v[nb, w * dd : w * (dd + 1)].rearrange("c p s -> p c s"),
                )
            xb = xbp.tile([128, CSUB, SEQ], bf16, tag="xb")
            # per-subtile cast so transposes can start on the first quarter
            for c in range(CSUB):
                nc.scalar.copy(xb[:, c, :], x_sb[:, c, :])

            # packed transposes: 2 per subtile, bf16 pairs ride f32 words
            # xt layout [p, h, c, r]; one psum bank covers 2 subtiles
            xt_sb = xtp.tile([128, 2, CSUB, 128], f32, tag="xt_sb")
            for d in range(CSUB // 2):
                xt_ps = ps_xt.tile([128, 2, 2, 128], f32, tag="xt_ps")
                for cc in range(2):
                    c = 2 * d + cc
                    xpk = xb[:, c, :].bitcast(f32)  # [128, 256] packed pairs
                    for h in range(2):
                        nc.tensor.transpose(
                            xt_ps[:, h, cc, :],
                            xpk[:, h * 128 : (h + 1) * 128],
                            ident[:, :],
                        )
                nc.scalar.copy(xt_sb[:, :, 2 * d : 2 * d + 2, :], xt_ps[:, :, :, :])

            # projection: 4 batched matmuls, N=512 (all subtiles at once)
            coeff_ps = ps_cf.tile([NB9, CSUB, 128], f32, tag="coeff_ps")
            for h in range(2):
                pair = (
                    xt_sb[:, h, :, :]
                    .bitcast(bf16)
                    .rearrange("v c (r q) -> v (c r) q", q=2)
                )
                for q in range(2):
                    nc.tensor.matmul(
                        coeff_ps[:, :, :],
                        cproj[:, h, q, :],
                        pair[:, :, q],
                        start=(h == 0 and q == 0),
                        stop=(h == 1 and q == 1),
                    )
            coeff_sb = cfp.tile([NB9, CSUB, 128], bf16, tag="coeff_sb")
            nc.vector.tensor_copy(coeff_sb[:, :, :], coeff_ps[:, :, :])

            resid_sb = rout.tile([128, CSUB, SEQ], f32, tag="resid_sb")
            fc_ps = ps_fc.tile([128, CSUB, PRED], f32, tag="fc_ps")
            for c in range(CSUB):
                trend_ps = ps_tr.tile([128, SEQ], f32, tag="trend_ps")
                nc.tensor.matmul(
                    trend_ps[:, :],
                    coeff_sb[:, c, :],
                    ctrec[:, :],
                    start=True,
                    stop=True,
                )
                nc.tensor.matmul(
                    fc_ps[:, c, :],
                    coeff_sb[:, c, :],
                    g_sb[:, :],
                    start=True,
                    stop=True,
                )
                nc.vector.tensor_sub(
                    out=resid_sb[:, c, :], in0=x_sb[:, c, :], in1=trend_ps[:, :]
                )
            # stores on GpSimd SWDGE: don't block the Sync load ring.
            # last block: per-subtile bias + stores so the final store chain
            # is as short as possible before the kernel drain barrier
            fc_sb = fout.tile([128, CSUB, PRED], f32, tag="fc_sb")
            if nb == nblk - 1:
                for c in range(CSUB):
                    nc.vector.tensor_add(
                        out=fc_sb[:, c, :], in0=fc_ps[:, c, :], in1=biasr[:, c, :]
                    )
                    nc.gpsimd.dma_start(out=resid_v[nb, c], in_=resid_sb[:, c, :])
                    nc.gpsimd.dma_start(out=fc_v[nb, :, c, :], in_=fc_sb[:, c, :])
            else:
                nc.vector.tensor_add(
                    out=fc_sb[:, :, :], in0=fc_ps[:, :, :], in1=biasr[:, :, :]
                )
                for dd in range(2):
                    nc.gpsimd.dma_start(
                        out=resid_v[nb, 2 * dd : 2 * dd + 2].rearrange(
                            "c p s -> p c s"
                        ),
                        in_=resid_sb[:, 2 * dd : 2 * dd + 2, :],
                    )
                nc.gpsimd.dma_start(out=fc_v[nb], in_=fc_sb[:, :, :])

    nc.finalize()
    return nc


def _get_nc():
    if "nc" not in _CACHE:
        _CACHE["nc"] = _build_nc()
    return _CACHE["nc"]


def _run(x, W, b, trace=False, **kwargs):
    from concourse.bass_utils import run_bass_kernel_spmd

    nc = _get_nc()
    consts = _host_consts(W, b)
    xs = x.reshape(NCORES, ROWS_PER_CORE, SEQ)
    in_maps = [dict(consts, x=np.ascontiguousarray(xs[i])) for i in range(NCORES)]
    res = run_bass_kernel_spmd(
        nc, in_maps, core_ids=list(range(NCORES)), trace=trace, **kwargs
    )
    fc = np.stack([r["fc"] for r in res.results]).reshape(B, NCH, PRED)
    resid = np.stack([r["resid"] for r in res.results]).reshape(B, NCH, SEQ)
    return (fc, resid), res


def kernel(**inputs):
    x = np.asarray(inputs["x"], dtype=np.float32)
    W = np.asarray(inputs["W"], dtype=np.float32)
    b = np.asarray(inputs["b"], dtype=np.float32)
    out, _ = _run(x, W, b)
    return out


# revision 32
# speedup vs baseline: 1.0940x; 1.0940x over previous
"""Trainium2 Bass kernel: FFT low-pass detrend + linear trend forecast.

reference math:
    X = rfft(x); keep bins 0..4; trend = irfft(X_low); resid = x - trend
    fc = trend @ W.T + b

Keeping the lowest 5 rfft bins is an orthogonal rank-9 projection:
    trend = (x @ C) @ C.T          C: [512, 9] = [DC, cos/sin k=1..4]
    fc    = (x @ C) @ (C.T @ W.T) + b
so the whole module is two tiny matmuls + a subtract. No FFT on device.

Sharding: pure data parallel over the batch dim across 8 NeuronCores.
Per core: 8192 rows x 512. Blocks of 512 rows (4x128-row subtiles):
  load x -> cast bf16 -> TensorE transpose-mode (bf16 pairs packed in
  f32 words: half the transpose instructions) -> batched N=512 bf16
  projection matmuls -> bf16 reconstruct + forecast matmuls -> DVE
  subtract -> stores.
Loads ride the Sync HWDGE ring; stores ride GpSimd SWDGE so slow
stores don't head-of-line-block the next block's load.
"""

import numpy as np

SEQ = 512
PRED = 96
NBINS = 5
NB9 = 2 * NBINS - 1  # 9 basis functions
NCORES = 8
B, NCH = 64, 1024
ROWS_PER_CORE = B * NCH // NCORES  # 8192
CSUB = 4  # 128-row subtiles per block
BLK_ROWS = 128 * CSUB  # 512
NBLK = ROWS_PER_CORE // BLK_ROWS  # 16

_CACHE = {}


def _basis_f64():
    s = np.arange(SEQ, dtype=np.float64)
    cols = [np.full(SEQ, 1.0 / np.sqrt(SEQ))]
    for k in range(1, NBINS):
        w = 2.0 * np.pi * k * s / SEQ
        cols.append(np.sqrt(2.0 / SEQ) * np.cos(w))
        cols.append(np.sqrt(2.0 / SEQ) * np.sin(w))
    return np.stack(cols, axis=1)  # [SEQ, 9]


def _host_consts(W, b):
    import ml_dtypes

    C = _basis_f64()
    v = np.arange(128)
    # cproj[v, h, q, j9] = C[2*(128h+v)+q, j9]  (packed-pair transpose layout)
    cproj = np.zeros((128, 2, 2, NB9), dtype=np.float64)
    for h in range(2):
        for q in range(2):
            cproj[:, h, q, :] = C[2 * (128 * h + v) + q, :]
    g = C.T @ W.astype(np.float64).T  # [9, 96]
    return {
        "cproj": cproj.astype(ml_dtypes.bfloat16),
        "ctrec": np.ascontiguousarray(C.T).astype(ml_dtypes.bfloat16),
        "g": g.astype(ml_dtypes.bfloat16),
        "biasr": np.ascontiguousarray(
            np.broadcast_to(b.astype(np.float32), (128, CSUB, PRED))
        ),
        "ident": np.eye(128, dtype=np.float32),
    }


def _build_nc(nblk=NBLK):
    from contextlib import ExitStack

    import concourse.mybir as mybir
    import concourse.tile as tile
    from concourse import bacc

    f32 = mybir.dt.float32
    bf16 = mybir.dt.bfloat16

    rows = nblk * BLK_ROWS
    nc = bacc.Bacc("TRN2", target_bir_lowering=False, debug=False)
    x_d = nc.dram_tensor("x", [rows, SEQ], f32, kind="ExternalInput")
    cproj_d = nc.dram_tensor("cproj", [128, 2, 2, NB9], bf16, kind="ExternalInput")
    ctrec_d = nc.dram_tensor("ctrec", [NB9, SEQ], bf16, kind="ExternalInput")
    g_d = nc.dram_tensor("g", [NB9, PRED], bf16, kind="ExternalInput")
    biasr_d = nc.dram_tensor("biasr", [128, CSUB, PRED], f32, kind="ExternalInput")
    ident_d = nc.dram_tensor("ident", [128, 128], f32, kind="ExternalInput")
    fc_d = nc.dram_tensor("fc", [rows, PRED], f32, kind="ExternalOutput")
    resid_d = nc.dram_tensor("resid", [rows, SEQ], f32, kind="ExternalOutput")

    # row r = nb*512 + c*128 + p
    x_v = x_d.ap().rearrange("(nb c p) s -> nb c p s", c=CSUB, p=128)
    resid_v = resid_d.ap().rearrange("(nb c p) s -> nb c p s", c=CSUB, p=128)
    fc_v = fc_d.ap().rearrange("(nb c p) j -> nb p c j", c=CSUB, p=128)

    with ExitStack() as ctx:
        tc = ctx.enter_context(tile.TileContext(nc))
        consts = ctx.enter_context(tc.tile_pool(name="consts", bufs=1))
        xin = ctx.enter_context(tc.tile_pool(name="xin", bufs=5))
        xbp = ctx.enter_context(tc.tile_pool(name="xbp", bufs=4))
        xtp = ctx.enter_context(tc.tile_pool(name="xtp", bufs=4))
        cfp = ctx.enter_context(tc.tile_pool(name="cfp", bufs=3))
        rout = ctx.enter_context(tc.tile_pool(name="rout", bufs=3))
        fout = ctx.enter_context(tc.tile_pool(name="fout", bufs=3))
        # PSUM budget: 8 banks. xt 3x1 + trend 2x1 + coeff 2x1 + fc 1x1 = 8
        ps_xt = ctx.enter_context(tc.tile_pool(name="ps_xt", bufs=3, space="PSUM"))
        ps_tr = ctx.enter_context(tc.tile_pool(name="ps_tr", bufs=2, space="PSUM"))
        ps_cf = ctx.enter_context(tc.tile_pool(name="ps_cf", bufs=2, space="PSUM"))
        ps_fc = ctx.enter_context(tc.tile_pool(name="ps_fc", bufs=1, space="PSUM"))

        # consts ride GpSimd SWDGE so the first x loads start immediately on Sync
        ident = consts.tile([128, 128], f32)
        nc.gpsimd.dma_start(out=ident[:, :], in_=ident_d.ap())
        cproj = consts.tile([128, 2, 2, NB9], bf16)
        nc.gpsimd.dma_start(out=cproj[:, :, :, :], in_=cproj_d.ap())
        ctrec = consts.tile([NB9, SEQ], bf16)
        nc.gpsimd.dma_start(out=ctrec[:, :], in_=ctrec_d.ap())
        g_sb = consts.tile([NB9, PRED], bf16)
        nc.gpsimd.dma_start(out=g_sb[:, :], in_=g_d.ap())
        biasr = consts.tile([128, CSUB, PRED], f32)
        nc.gpsimd.dma_start(out=biasr[:, :, :], in_=biasr_d.ap())

        for nb in range(nblk):
            # loads: two 512KiB DMAs per block (block 0: four 256KiB to cut
            # the time-to-first-compute at kernel start)
            x_sb = xin.tile([128, CSUB, SEQ], f32, tag="x_sb")
            nld = 4 if nb == 0 else 2
            for dd in range(nld):
                w = CSUB // nld
                nc.sync.dma_start(
                    out=x_sb[:, w * dd : w * (dd + 1), :],
                    in_=x_v[nb, w * dd : w * (dd + 1)].rearrange("c p s -> p c s"),
                )
            xb = xbp.tile([128, CSUB, SEQ], bf16, tag="xb")
            # per-subtile cast so transposes can start on the first quarter
            for c in range(CSUB):
                nc.scalar.copy(xb[:, c, :], x_sb[:, c, :])

            # packed transposes: 2 per subtile, bf16 pairs ride f32 words
            # xt layout [p, h, c, r]; one psum bank covers 2 subtiles
            xt_sb = xtp.tile([128, 2, CSUB, 128], f32, tag="xt_sb")
            for d in range(CSUB // 2):
                xt_ps = ps_xt.tile([128, 2, 2, 128], f32, tag="xt_ps")
                for cc in range(2):
                    c = 2 * d + cc
                    xpk = xb[:, c, :].bitcast(f32)  # [128, 256] packed pairs
                    for h in range(2):
                        nc.tensor.transpose(
                            xt_ps[:, h, cc, :],
                            xpk[:, h * 128 : (h + 1) * 128],
                            ident[:, :],
                        )
                nc.scalar.copy(xt_sb[:, :, 2 * d : 2 * d + 2, :], xt_ps[:, :, :, :])

            # projection: 4 batched matmuls, N=512 (all subtiles at once)
            coeff_ps = ps_cf.tile([NB9, CSUB, 128], f32, tag="coeff_ps")
            for h in range(2):
                pair = (
                    xt_sb[:, h, :, :]
                    .bitcast(bf16)
                    .rearrange("v c (r q) -> v (c r) q", q=2)
                )
                for q in range(2):
                    nc.tensor.matmul(
                        coeff_ps[:, :, :],
                        cproj[:, h, q, :],
                        pair[:, :, q],
                        start=(h == 0 and q == 0),
                        stop=(h == 1 and q == 1),
                    )
            coeff_sb = cfp.tile([NB9, CSUB, 128], bf16, tag="coeff_sb")
            nc.vector.tensor_copy(coeff_sb[:, :, :], coeff_ps[:, :, :])

            resid_sb = rout.tile([128, CSUB, SEQ], f32, tag="resid_sb")
            fc_ps = ps_fc.tile([128, CSUB, PRED], f32, tag="fc_ps")
            for c in range(CSUB):
                trend_ps = ps_tr.tile([128, SEQ], f32, tag="trend_ps")
                nc.tensor.matmul(
                    trend_ps[:, :],
                    coeff_sb[:, c, :],
                    ctrec[:, :],
                    start=True,
                    stop=True,
                )
                nc.tensor.matmul(
                    fc_ps[:, c, :],
                    coeff_sb[:, c, :],
                    g_sb[:, :],
                    start=True,
                    stop=True,
                )
                nc.vector.tensor_sub(
                    out=resid_sb[:, c, :], in0=x_sb[:, c, :], in1=trend_ps[:, :]
                )
            # stores on GpSimd SWDGE: don't block the Sync load ring.
            # last block: per-subtile bias + stores so the final store chain
            # is as short as possible before the kernel drain barrier
            fc_sb = fout.tile([128, CSUB, PRED], f32, tag="fc_sb")
            if nb == nblk - 1:
                for c in range(CSUB):
                    nc.vector.tensor_add(
                        out=fc_sb[:, c, :], in0=fc_ps[:, c, :], in1=biasr[:, c, :]
                    )
                    nc.gpsimd.dma_start(out=resid_v[nb, c], in_=resid_sb[:, c, :])
                    nc.gpsimd.dma_start(out=fc_v[nb, :, c, :], in_=fc_sb[:, c, :])
            else:
                nc.vector.tensor_add(
                    out=fc_sb[:, :, :], in0=fc_ps[:, :, :], in1=biasr[:, :, :]
                )
                for dd in range(2):
                    nc.gpsimd.dma_start(
                        out=resid_v[nb, 2 * dd : 2 * dd + 2].rearrange(
                            "c p s -> p c s"
                        ),
                        in_=resid_sb[:, 2 * dd : 2 * dd + 2, :],
                    )
                nc.gpsimd.dma_start(out=fc_v[nb], in_=fc_sb[:, :, :])

    nc.finalize()
    return nc


def _get_nc():
    if "nc" not in _CACHE:
        _CACHE["nc"] = _build_nc()
    return _CACHE["nc"]


def _run(x, W, b, trace=False, **kwargs):
    from concourse.bass_utils import run_bass_kernel_spmd

    nc = _get_nc()
    consts = _host_consts(W, b)
    xs = x.reshape(NCORES, ROWS_PER_CORE, SEQ)
    in_maps = [dict(consts, x=np.ascontiguousarray(xs[i])) for i in range(NCORES)]
    res = run_bass_kernel_spmd(
        nc, in_maps, core_ids=list(range(NCORES)), trace=trace, **kwargs
    )
    fc = np.stack([r["fc"] for r in res.results]).reshape(B, NCH, PRED)
    resid = np.stack([r["resid"] for r in res.results]).reshape(B, NCH, SEQ)
    return (fc, resid), res


def kernel(**inputs):
    x = np.asarray(inputs["x"], dtype=np.float32)
    W = np.asarray(inputs["W"], dtype=np.float32)
    b = np.asarray(inputs["b"], dtype=np.float32)
    out, _ = _run(x, W, b)
    return out
